# revision 12
# baseline (speedup 1.0000x reference)
"""Contrastive-loss kernel for Trainium2 (8 NeuronCores, Bass/Tile).

Math (reference):
    W = wsi[:, 0, :], O = omic[:, 0, :]                      # [N, D]
    S = (W @ O.T) / max(|W_i||O_j|, eps)                     # [N, N] cosine sims
    d = diag(S)
    L = where(eye, 1 - S, relu(M - S + d[:, None]))
    out = mean(L)

Identity used on device: relu(M - S_ii + d_i) == M exactly, so
    sum(L) = sum_{i,j} relu(M - S_ij + d_i) + sum_i (1 - d_i - M)
which needs no diagonal masking of the big [N, N] block.

Distribution: data-parallel over W rows. Each core c gets its 512 W rows
(pre-normalized, pre-transposed to [D, rows] layout, bf16) plus the full
normalized O, transposed and column-rotated by 512*c so the diagonal block
always lands in j-chunk 0 (keeps the SPMD program core-independent). Each
core computes its [512, 4096] block of S on the PE (bf16 in, fp32 psum),
applies the hinge + row-sum on the Scalar engine, and DMAs out a [128, 1]
partial sum. Host adds the 8 partials and divides by N^2.
"""

import numpy as np
import ml_dtypes

N = 4096
D = 1024
NCORES = 8
ROWS = N // NCORES  # 512 W rows per core
P = 128             # SBUF partitions
NJ = 512            # moving free dim per matmul (one PSUM bank of fp32)
TI = ROWS // P      # 4 i-tiles per core
ND = D // P         # 8 contraction chunks
NJC = N // NJ       # 8 j-chunks
MARGIN = 0.1
N_WARMUP = 20      # wide PE-warmup matmuls issued while DMAs stream

_cache = {}


def _build():
    from contextlib import ExitStack
    import concourse.bacc as bacc
    import concourse.tile as tile
    import concourse.mybir as mybir

    f32 = mybir.dt.float32
    bf16 = mybir.dt.bfloat16

    nc = bacc.Bacc("TRN2", target_bir_lowering=False, debug=False,
                   num_devices=NCORES)
    wt_d = nc.dram_tensor("wt", [P, TI * ND * P], bf16, kind="ExternalInput").ap()
    ot_d = nc.dram_tensor("ot", [P, NJC * ND * NJ], bf16, kind="ExternalInput").ap()
    id_d = nc.dram_tensor("id", [P, P], f32, kind="ExternalInput").ap()
    out_d = nc.dram_tensor("out", [1, TI * (NJC + 1)], f32,
                           kind="ExternalOutput").ap()

    with tile.TileContext(nc) as tc, ExitStack() as ctx:
        const = ctx.enter_context(tc.tile_pool(name="const", bufs=1))
        otp = ctx.enter_context(tc.tile_pool(name="otp", bufs=NJC))
        pp = ctx.enter_context(tc.tile_pool(name="pp", bufs=4, space="PSUM"))
        pp1 = ctx.enter_context(tc.tile_pool(name="pp1", bufs=1, space="PSUM"))
        scrp = ctx.enter_context(tc.tile_pool(name="scr", bufs=4))
        smallp = ctx.enter_context(tc.tile_pool(name="small", bufs=2))

        # DMA order puts the first matmul's operands (t=0 weights + j-chunk 0)
        # at the head of the HWDGE queue; everything else streams behind.
        wt_sb = const.tile([P, TI * ND * P], bf16, tag="wt")
        nc.sync.dma_start(out=wt_sb[:, 0:ND * P], in_=wt_d[:, 0:ND * P])
        ots = []
        o0 = otp.tile([P, ND * NJ], bf16, tag="ot")
        nc.sync.dma_start(out=o0[:], in_=ot_d[:, 0:ND * NJ])
        ots.append(o0)
        nc.sync.dma_start(out=wt_sb[:, ND * P:], in_=wt_d[:, ND * P:])
        id_sb = const.tile([P, P], f32, tag="id")
        nc.sync.dma_start(out=id_sb[:], in_=id_d[:, :])
        for jc in range(1, NJC):
            o = otp.tile([P, ND * NJ], bf16, tag="ot")
            nc.sync.dma_start(out=o[:], in_=ot_d[:, jc * ND * NJ:(jc + 1) * ND * NJ])
            ots.append(o)
        ones_sb = const.tile([P, 1], f32, tag="ones")
        nc.vector.memset(ones_sb[:], 1.0)

        # Warm the PE clock (HAM gate releases after ~3.4us of sustained
        # array activity) while the first DMAs stream: wide N=512 matmuls
        # on a memset tile keep the array at full duty so the real matmul
        # stream starts at 2.4 GHz instead of 1.2 GHz.
        warm_rhs = const.tile([P, NJ], f32, tag="warmrhs")
        nc.vector.memset(warm_rhs[:], 0.0)
        warm_ps = pp1.tile([1, NJ], f32, tag="warmps")
        for _ in range(N_WARMUP):
            nc.tensor.matmul(warm_ps[:], lhsT=ones_sb[:], rhs=warm_rhs[:],
                             start=True, stop=True)

        # per-(t,jc) hinge row-sums + per-t corrections, reduced at the end
        acc = const.tile([P, TI * (NJC + 1)], f32, tag="acc")
        hb = const.tile([P, TI], f32, tag="hb")  # hb[:, t] = MARGIN + d_i

        for jc in range(NJC):
            for t in range(TI):
                ps = pp.tile([P, NJ], f32, tag="ps")
                for d in range(ND):
                    nc.tensor.matmul(
                        ps[:],
                        lhsT=wt_sb[:, (t * ND + d) * P:(t * ND + d + 1) * P],
                        rhs=ots[jc][:, d * NJ:(d + 1) * NJ],
                        start=(d == 0),
                        stop=(d == ND - 1),
                    )
                if jc == 0:
                    # hb[:, t] = MARGIN + d_i  (diag of this block via identity
                    # mask; tensor_tensor_reduce faults the exec unit on this
                    # runtime, so use mul + reduce + bias-add instead)
                    dprod = scrp.tile([P, P], f32, tag="dprod")
                    nc.vector.tensor_mul(
                        dprod[:], ps[:, t * P:(t + 1) * P], id_sb[:])
                    dsum = scrp.tile([P, 1], f32, tag="dsum")
                    nc.vector.tensor_reduce(
                        out=dsum[:], in_=dprod[:],
                        axis=mybir.AxisListType.X, op=mybir.AluOpType.add)
                    nc.scalar.activation(
                        out=hb[:, t:t + 1], in_=dsum[:],
                        func=mybir.ActivationFunctionType.Copy,
                        bias=MARGIN, scale=1.0)
                    # correction column: 1 - d_i - MARGIN = 1 - hb
                    nc.scalar.activation(
                        out=acc[:, t * (NJC + 1) + NJC:t * (NJC + 1) + NJC + 1],
                        in_=hb[:, t:t + 1],
                        func=mybir.ActivationFunctionType.Copy,
                        bias=1.0,
                        scale=-1.0,
                    )
                h = scrp.tile([P, NJ], f32, tag="h")
                nc.scalar.activation(
                    out=h[:],
                    in_=ps[:],
                    func=mybir.ActivationFunctionType.Relu,
                    bias=hb[:, t:t + 1],
                    scale=-1.0,
                    accum_out=acc[:, t * (NJC + 1) + jc:t * (NJC + 1) + jc + 1],
                )

        # cross-partition reduce on the PE (ones^T @ acc -> [1, 36]) so the
        # output DMA is one contiguous partition line instead of 128 4-byte
        # descriptors (whose completion receipts dominate the kernel tail)
        tot_ps = pp1.tile([1, TI * (NJC + 1)], f32, tag="totps")
        nc.tensor.matmul(tot_ps[:], lhsT=ones_sb[:], rhs=acc[:, :],
                         start=True, stop=True)
        total = smallp.tile([1, TI * (NJC + 1)], f32, tag="tot")
        nc.vector.tensor_copy(total[:], tot_ps[:])
        nc.sync.dma_start(out=out_d[:, :], in_=total[:])

    nc.compile()
    return nc


def _get_nc():
    if "nc" not in _cache:
        _cache["nc"] = _build()
    return _cache["nc"]


def _prep_inputs(wsi, omic):
    W = np.asarray(wsi, dtype=np.float32)[:, 0, :].astype(np.float64)
    O = np.asarray(omic, dtype=np.float32)[:, 0, :].astype(np.float64)
    Wn = (W / np.maximum(np.linalg.norm(W, axis=1, keepdims=True), 1e-30))
    On = (O / np.maximum(np.linalg.norm(O, axis=1, keepdims=True), 1e-30))
    Wn = Wn.astype(ml_dtypes.bfloat16)
    On = On.astype(ml_dtypes.bfloat16)
    ident = np.eye(P, dtype=np.float32)

    in_maps = []
    for c in range(NCORES):
        Wc = Wn[c * ROWS:(c + 1) * ROWS]  # [512, 1024]
        # wt[k, (t*ND + d)*P + m] = Wc[t*P + m, d*P + k]
        wt = np.ascontiguousarray(
            Wc.reshape(TI, P, ND, P).transpose(3, 0, 2, 1).reshape(P, TI * ND * P))
        # column rotation: permuted col j' <-> original O row (j' + 512c) % N
        Operm = np.roll(On, -ROWS * c, axis=0)
        # ot[k, (jc*ND + d)*NJ + n] = Operm[jc*NJ + n, d*P + k]
        ot = np.ascontiguousarray(
            Operm.reshape(NJC, NJ, ND, P).transpose(3, 0, 2, 1)
            .reshape(P, NJC * ND * NJ))
        in_maps.append({"wt": wt, "ot": ot, "id": ident})
    return in_maps


def kernel(wsi_embeddings, omic_embeddings):
    from concourse.bass_utils import run_bass_kernel_spmd

    nc = _get_nc()
    in_maps = _prep_inputs(wsi_embeddings, omic_embeddings)
    res = run_bass_kernel_spmd(nc, in_maps, list(range(NCORES)))
    grand = 0.0
    for c in range(NCORES):
        grand += res.results[c]["out"].astype(np.float64).sum()
    return np.float32(grand / (float(N) * float(N)))


# revision 14
# speedup vs baseline: 1.1429x; 1.1429x over previous
"""Contrastive-loss kernel for Trainium2 (8 NeuronCores, Bass/Tile).

Math (reference):
    W = wsi[:, 0, :], O = omic[:, 0, :]                      # [N, D]
    S = (W @ O.T) / max(|W_i||O_j|, eps)                     # [N, N] cosine sims
    d = diag(S)
    L = where(eye, 1 - S, relu(M - S + d[:, None]))
    out = mean(L)

Identity used on device: relu(M - S_ii + d_i) == M exactly, so
    sum(L) = sum_{i,j} relu(M - S_ij + d_i) + sum_i (1 - d_i - M)
which needs no diagonal masking of the big [N, N] block.

Distribution: data-parallel over W rows. Each core c gets its 512 W rows
(pre-normalized, pre-transposed to [D, rows] layout, bf16) plus the full
normalized O, transposed and column-rotated by 512*c so the diagonal block
always lands in j-chunk 0 (keeps the SPMD program core-independent). Each
core computes its [512, 4096] block of S on the PE (bf16 in, fp32 psum),
applies the hinge + row-sum on the Scalar engine, and DMAs out a [128, 1]
partial sum. Host adds the 8 partials and divides by N^2.
"""

import numpy as np
import ml_dtypes

N = 4096
D = 1024
NCORES = 8
ROWS = N // NCORES  # 512 W rows per core
P = 128             # SBUF partitions
NJ = 512            # moving free dim per matmul (one PSUM bank of fp32)
TI = ROWS // P      # 4 i-tiles per core
ND = D // P         # 8 contraction chunks
NJC = N // NJ       # 8 j-chunks
MARGIN = 0.1
N_WARMUP = 24      # wide bf16 PE-warmup matmuls issued while DMAs stream

_cache = {}


def _build():
    from contextlib import ExitStack
    import concourse.bacc as bacc
    import concourse.tile as tile
    import concourse.mybir as mybir

    f32 = mybir.dt.float32
    bf16 = mybir.dt.bfloat16

    nc = bacc.Bacc("TRN2", target_bir_lowering=False, debug=False,
                   num_devices=NCORES)
    wt_d = nc.dram_tensor("wt", [P, TI * ND * P], bf16, kind="ExternalInput").ap()
    ot_d = nc.dram_tensor("ot", [P, NJC * ND * NJ], bf16, kind="ExternalInput").ap()
    id_d = nc.dram_tensor("id", [P, P], f32, kind="ExternalInput").ap()
    out_d = nc.dram_tensor("out", [1, TI * (NJC + 1)], f32,
                           kind="ExternalOutput").ap()

    with tile.TileContext(nc) as tc, ExitStack() as ctx:
        const = ctx.enter_context(tc.tile_pool(name="const", bufs=1))
        otp = ctx.enter_context(tc.tile_pool(name="otp", bufs=NJC))
        pp = ctx.enter_context(tc.tile_pool(name="pp", bufs=4, space="PSUM"))
        pp1 = ctx.enter_context(tc.tile_pool(name="pp1", bufs=1, space="PSUM"))
        scrp = ctx.enter_context(tc.tile_pool(name="scr", bufs=4))
        smallp = ctx.enter_context(tc.tile_pool(name="small", bufs=2))

        # DMA order puts the first matmul's operands (t=0 weights + j-chunk 0)
        # at the head of the HWDGE queue; everything else streams behind.
        wt_sb = const.tile([P, TI * ND * P], bf16, tag="wt")
        nc.sync.dma_start(out=wt_sb[:, 0:ND * P], in_=wt_d[:, 0:ND * P])
        ots = []
        o0 = otp.tile([P, ND * NJ], bf16, tag="ot")
        nc.sync.dma_start(out=o0[:], in_=ot_d[:, 0:ND * NJ])
        ots.append(o0)
        nc.sync.dma_start(out=wt_sb[:, ND * P:], in_=wt_d[:, ND * P:])
        id_sb = const.tile([P, P], f32, tag="id")
        nc.sync.dma_start(out=id_sb[:], in_=id_d[:, :])
        for jc in range(1, NJC):
            o = otp.tile([P, ND * NJ], bf16, tag="ot")
            nc.sync.dma_start(out=o[:], in_=ot_d[:, jc * ND * NJ:(jc + 1) * ND * NJ])
            ots.append(o)
        ones_sb = const.tile([P, 1], f32, tag="ones")
        nc.vector.memset(ones_sb[:], 1.0)

        # Warm the PE clock (HAM gate releases after ~3.4us of sustained
        # array activity) while the first DMAs stream: wide N=512 matmuls
        # on a memset tile keep the array at full duty so the real matmul
        # stream starts at 2.4 GHz instead of 1.2 GHz.
        warm_w = const.tile([P, 1], bf16, tag="warmw")
        nc.vector.memset(warm_w[:], 0.0)
        warm_rhs = const.tile([P, NJ], bf16, tag="warmrhs")
        nc.vector.memset(warm_rhs[:], 0.0)
        warm_ps = pp1.tile([1, NJ], f32, tag="warmps")
        for _ in range(N_WARMUP):
            nc.tensor.matmul(warm_ps[:], lhsT=warm_w[:], rhs=warm_rhs[:],
                             start=True, stop=True)

        # per-(t,jc) hinge row-sums + per-t corrections, reduced at the end
        acc = const.tile([P, TI * (NJC + 1)], f32, tag="acc")
        hb = const.tile([P, TI], f32, tag="hb")  # hb[:, t] = MARGIN + d_i

        for jc in range(NJC):
            for t in range(TI):
                ps = pp.tile([P, NJ], f32, tag="ps")
                for d in range(ND):
                    nc.tensor.matmul(
                        ps[:],
                        lhsT=wt_sb[:, (t * ND + d) * P:(t * ND + d + 1) * P],
                        rhs=ots[jc][:, d * NJ:(d + 1) * NJ],
                        start=(d == 0),
                        stop=(d == ND - 1),
                    )
                if jc == 0:
                    # hb[:, t] = MARGIN + d_i  (diag of this block via identity
                    # mask; tensor_tensor_reduce faults the exec unit on this
                    # runtime, so use mul + reduce + bias-add instead)
                    dprod = scrp.tile([P, P], f32, tag="dprod")
                    nc.vector.tensor_mul(
                        dprod[:], ps[:, t * P:(t + 1) * P], id_sb[:])
                    dsum = scrp.tile([P, 1], f32, tag="dsum")
                    nc.vector.tensor_reduce(
                        out=dsum[:], in_=dprod[:],
                        axis=mybir.AxisListType.X, op=mybir.AluOpType.add)
                    nc.scalar.activation(
                        out=hb[:, t:t + 1], in_=dsum[:],
                        func=mybir.ActivationFunctionType.Copy,
                        bias=MARGIN, scale=1.0)
                    # correction column: 1 - d_i - MARGIN = 1 - hb
                    nc.scalar.activation(
                        out=acc[:, t * (NJC + 1) + NJC:t * (NJC + 1) + NJC + 1],
                        in_=hb[:, t:t + 1],
                        func=mybir.ActivationFunctionType.Copy,
                        bias=1.0,
                        scale=-1.0,
                    )
                h = scrp.tile([P, NJ], f32, tag="h")
                nc.scalar.activation(
                    out=h[:],
                    in_=ps[:],
                    func=mybir.ActivationFunctionType.Relu,
                    bias=hb[:, t:t + 1],
                    scale=-1.0,
                    accum_out=acc[:, t * (NJC + 1) + jc:t * (NJC + 1) + jc + 1],
                )

        # cross-partition reduce on the PE (ones^T @ acc -> [1, 36]) so the
        # output DMA is one contiguous partition line instead of 128 4-byte
        # descriptors (whose completion receipts dominate the kernel tail)
        tot_ps = pp1.tile([1, TI * (NJC + 1)], f32, tag="totps")
        nc.tensor.matmul(tot_ps[:], lhsT=ones_sb[:], rhs=acc[:, :],
                         start=True, stop=True)
        total = smallp.tile([1, TI * (NJC + 1)], f32, tag="tot")
        nc.vector.tensor_copy(total[:], tot_ps[:])
        nc.sync.dma_start(out=out_d[:, :], in_=total[:])

    nc.compile()
    return nc


def _get_nc():
    if "nc" not in _cache:
        _cache["nc"] = _build()
    return _cache["nc"]


def _prep_inputs(wsi, omic):
    W = np.asarray(wsi, dtype=np.float32)[:, 0, :].astype(np.float64)
    O = np.asarray(omic, dtype=np.float32)[:, 0, :].astype(np.float64)
    Wn = (W / np.maximum(np.linalg.norm(W, axis=1, keepdims=True), 1e-30))
    On = (O / np.maximum(np.linalg.norm(O, axis=1, keepdims=True), 1e-30))
    Wn = Wn.astype(ml_dtypes.bfloat16)
    On = On.astype(ml_dtypes.bfloat16)
    ident = np.eye(P, dtype=np.float32)

    in_maps = []
    for c in range(NCORES):
        Wc = Wn[c * ROWS:(c + 1) * ROWS]  # [512, 1024]
        # wt[k, (t*ND + d)*P + m] = Wc[t*P + m, d*P + k]
        wt = np.ascontiguousarray(
            Wc.reshape(TI, P, ND, P).transpose(3, 0, 2, 1).reshape(P, TI * ND * P))
        # column rotation: permuted col j' <-> original O row (j' + 512c) % N
        Operm = np.roll(On, -ROWS * c, axis=0)
        # ot[k, (jc*ND + d)*NJ + n] = Operm[jc*NJ + n, d*P + k]
        ot = np.ascontiguousarray(
            Operm.reshape(NJC, NJ, ND, P).transpose(3, 0, 2, 1)
            .reshape(P, NJC * ND * NJ))
        in_maps.append({"wt": wt, "ot": ot, "id": ident})
    return in_maps


def kernel(wsi_embeddings, omic_embeddings):
    from concourse.bass_utils import run_bass_kernel_spmd

    nc = _get_nc()
    in_maps = _prep_inputs(wsi_embeddings, omic_embeddings)
    res = run_bass_kernel_spmd(nc, in_maps, list(range(NCORES)))
    grand = 0.0
    for c in range(NCORES):
        grand += res.results[c]["out"].astype(np.float64).sum()
    return np.float32(grand / (float(N) * float(N)))


# revision 15
# speedup vs baseline: 1.8761x; 1.6415x over previous
"""Contrastive-loss kernel for Trainium2 (8 NeuronCores, Bass/Tile).

Math (reference):
    W = wsi[:, 0, :], O = omic[:, 0, :]                      # [N, D]
    S = (W @ O.T) / max(|W_i||O_j|, eps)                     # [N, N] cosine sims
    d = diag(S)
    L = where(eye, 1 - S, relu(M - S + d[:, None]))
    out = mean(L)

Device identity (no diagonal masking of the [N, N] block needed):
    sum(L) = sum_{i,j} relu(hb_i - S_ij) + sum_i [(1 - d_i) - relu(hb_i - S_ii)]
    with hb_i = M + d_i. Since hb_i - S_ii ~= M > 0, the per-row correction is
    (1 + M - 2 d_i) + S_ii: the device only ships row-sums of relu(hb - S) and
    the diagonal entries S_ii; the analytic part is added on the host.

Distribution: data-parallel over W rows. Each core c gets its 512 W rows
(pre-normalized, fp8-e4m3, DoubleRow-packed) plus the full normalized O,
column-rotated by 512*c so the diagonal block always lands in j-chunk 0
(keeps the SPMD program core-independent). The exact diagonal bias hb is
computed on the host in f64 and shipped, which removes the row-correlated
part of the fp8 quantization error. Each core computes its [512, 4096]
block of S on the PE (fp8 DoubleRow, fp32 psum), the Scalar engine applies
the hinge, the Vector engine row-sums it, and one ones-matmul collapses
partitions so the output DMA is a single 144-byte partition line.
"""

import numpy as np
import ml_dtypes

N = 4096
D = 1024
NCORES = 8
ROWS = N // NCORES  # 512 W rows per core
P = 128             # SBUF partitions
NJ = 512            # moving free dim per matmul (one PSUM bank of fp32)
TI = ROWS // P      # 4 i-tiles per core
ND2 = D // 256      # 4 DoubleRow contraction chunks (256 deep each)
NJC = N // NJ       # 8 j-chunks
MARGIN = 0.1
N_WARMUP = 18       # wide bf16 PE-warmup matmuls issued while DMAs stream
NCOL = TI * (NJC + 1)  # acc columns: per t, 8 hinge row-sums + 1 diag column

_cache = {}


def _build():
    from contextlib import ExitStack
    import concourse.bacc as bacc
    import concourse.tile as tile
    import concourse.mybir as mybir

    f32 = mybir.dt.float32
    bf16 = mybir.dt.bfloat16
    fp8 = mybir.dt.float8e4

    nc = bacc.Bacc("TRN2", target_bir_lowering=False, debug=False,
                   num_devices=NCORES)
    wt_d = nc.dram_tensor("wt", [P, TI * ND2, 2, P], fp8,
                          kind="ExternalInput").ap()
    ot_d = nc.dram_tensor("ot", [P, NJC * ND2, 2, NJ], fp8,
                          kind="ExternalInput").ap()
    id_d = nc.dram_tensor("id", [P, P], f32, kind="ExternalInput").ap()
    hb_d = nc.dram_tensor("hb", [P, TI], f32, kind="ExternalInput").ap()
    out_d = nc.dram_tensor("out", [1, NCOL], f32, kind="ExternalOutput").ap()

    with tile.TileContext(nc) as tc, ExitStack() as ctx:
        const = ctx.enter_context(tc.tile_pool(name="const", bufs=1))
        otp = ctx.enter_context(tc.tile_pool(name="otp", bufs=NJC))
        pp = ctx.enter_context(tc.tile_pool(name="pp", bufs=4, space="PSUM"))
        pp1 = ctx.enter_context(tc.tile_pool(name="pp1", bufs=1, space="PSUM"))
        scrp = ctx.enter_context(tc.tile_pool(name="scr", bufs=4))
        smallp = ctx.enter_context(tc.tile_pool(name="small", bufs=2))

        # DMA order puts the first matmul's operands (t=0 weights + j-chunk 0)
        # at the head of the HWDGE queue; everything else streams behind.
        wt_sb = const.tile([P, TI * ND2, 2, P], fp8, tag="wt")
        nc.sync.dma_start(out=wt_sb[:, 0:ND2, :, :], in_=wt_d[:, 0:ND2, :, :])
        ots = []
        o0 = otp.tile([P, ND2, 2, NJ], fp8, tag="ot")
        nc.sync.dma_start(out=o0[:], in_=ot_d[:, 0:ND2, :, :])
        ots.append(o0)
        nc.sync.dma_start(out=wt_sb[:, ND2:, :, :], in_=wt_d[:, ND2:, :, :])
        hb = const.tile([P, TI], f32, tag="hb")
        nc.sync.dma_start(out=hb[:], in_=hb_d[:, :])
        id_sb = const.tile([P, P], f32, tag="id")
        nc.sync.dma_start(out=id_sb[:], in_=id_d[:, :])
        for jc in range(1, NJC):
            o = otp.tile([P, ND2, 2, NJ], fp8, tag="ot")
            nc.sync.dma_start(out=o[:],
                              in_=ot_d[:, jc * ND2:(jc + 1) * ND2, :, :])
            ots.append(o)
        ones_sb = const.tile([P, 1], f32, tag="ones")
        nc.vector.memset(ones_sb[:], 1.0)

        # Warm the PE clock (HAM gate releases after ~3.4us of sustained
        # array activity) while the first DMAs stream, so the real matmul
        # stream starts at 2.4 GHz instead of 1.2 GHz.
        warm_w = const.tile([P, 1], bf16, tag="warmw")
        nc.vector.memset(warm_w[:], 0.0)
        warm_rhs = const.tile([P, NJ], bf16, tag="warmrhs")
        nc.vector.memset(warm_rhs[:], 0.0)
        warm_ps = pp1.tile([1, NJ], f32, tag="warmps")
        for _ in range(N_WARMUP):
            nc.tensor.matmul(warm_ps[:], lhsT=warm_w[:], rhs=warm_rhs[:],
                             start=True, stop=True)

        # per-(t,jc) hinge row-sums + per-t diagonal column
        acc = const.tile([P, NCOL], f32, tag="acc")

        for jc in range(NJC):
            for t in range(TI):
                ps = pp.tile([P, NJ], f32, tag="ps")
                for dd in range(ND2):
                    nc.tensor.matmul(
                        ps[:],
                        lhsT=wt_sb[:, t * ND2 + dd, :, :],
                        rhs=ots[jc][:, dd, :, :],
                        start=(dd == 0),
                        stop=(dd == ND2 - 1),
                        perf_mode=mybir.MatmulPerfMode.DoubleRow,
                    )
                if jc == 0:
                    # acc diag column = S_ii (diag of this block via identity
                    # mask; tensor_tensor_reduce faults the exec unit on this
                    # runtime, so mask + reduce in two DVE ops)
                    dprod = scrp.tile([P, P], f32, tag="dprod")
                    nc.vector.tensor_mul(
                        dprod[:], ps[:, t * P:(t + 1) * P], id_sb[:])
                    nc.vector.tensor_reduce(
                        out=acc[:, t * (NJC + 1) + NJC:t * (NJC + 1) + NJC + 1],
                        in_=dprod[:],
                        axis=mybir.AxisListType.X, op=mybir.AluOpType.add)
                # hinge on ACT (bf16 out halves the DVE reduce cost), row-sum
                # on DVE; together they stay under the PE's block time
                h = scrp.tile([P, NJ], bf16, tag="h")
                nc.scalar.activation(
                    out=h[:],
                    in_=ps[:],
                    func=mybir.ActivationFunctionType.Relu,
                    bias=hb[:, t:t + 1],
                    scale=-1.0,
                )
                nc.vector.tensor_reduce(
                    out=acc[:, t * (NJC + 1) + jc:t * (NJC + 1) + jc + 1],
                    in_=h[:],
                    axis=mybir.AxisListType.X, op=mybir.AluOpType.add)

        # cross-partition reduce on the PE (ones^T @ acc -> [1, 36]) so the
        # output DMA is one contiguous partition line instead of 128 4-byte
        # descriptors (whose completion receipts dominate the kernel tail)
        tot_ps = pp1.tile([1, NCOL], f32, tag="totps")
        nc.tensor.matmul(tot_ps[:], lhsT=ones_sb[:], rhs=acc[:, :],
                         start=True, stop=True)
        total = smallp.tile([1, NCOL], f32, tag="tot")
        nc.vector.tensor_copy(total[:], tot_ps[:])
        nc.sync.dma_start(out=out_d[:, :], in_=total[:])

    nc.compile()
    return nc


def _get_nc():
    if "nc" not in _cache:
        _cache["nc"] = _build()
    return _cache["nc"]


def _prep_inputs(wsi, omic):
    fp8np = ml_dtypes.float8_e4m3
    W = np.asarray(wsi, dtype=np.float32)[:, 0, :].astype(np.float64)
    O = np.asarray(omic, dtype=np.float32)[:, 0, :].astype(np.float64)
    Wn = W / np.maximum(np.linalg.norm(W, axis=1, keepdims=True), 1e-30)
    On = O / np.maximum(np.linalg.norm(O, axis=1, keepdims=True), 1e-30)
    d_exact = np.einsum("nd,nd->n", Wn, On)  # exact cos(w_i, o_i)
    hb_all = (MARGIN + d_exact).astype(np.float32)
    Wn8 = Wn.astype(fp8np)
    On8 = On.astype(fp8np)
    ident = np.eye(P, dtype=np.float32)

    in_maps = []
    for c in range(NCORES):
        Wc = Wn8[c * ROWS:(c + 1) * ROWS]  # [512, 1024]
        # wt[p, t*ND2+dd, r, m] = Wc[t*128+m, dd*256 + r*128 + p]
        wt = np.ascontiguousarray(
            Wc.reshape(TI, P, ND2, 2, P).transpose(4, 0, 2, 3, 1)
            .reshape(P, TI * ND2, 2, P))
        # column rotation: permuted col j' <-> original O row (j' + 512c) % N
        Operm = np.roll(On8, -ROWS * c, axis=0)
        # ot[p, jc*ND2+dd, r, n] = Operm[jc*512 + n, dd*256 + r*128 + p]
        ot = np.ascontiguousarray(
            Operm.reshape(NJC, NJ, ND2, 2, P).transpose(4, 0, 2, 3, 1)
            .reshape(P, NJC * ND2, 2, NJ))
        # hb[p, t] = MARGIN + d_exact[c*512 + t*128 + p]
        hbc = np.ascontiguousarray(
            hb_all[c * ROWS:(c + 1) * ROWS].reshape(TI, P).T)
        in_maps.append({"wt": wt, "ot": ot, "id": ident, "hb": hbc})
    return in_maps, d_exact


def kernel(wsi_embeddings, omic_embeddings):
    from concourse.bass_utils import run_bass_kernel_spmd

    nc = _get_nc()
    in_maps, d_exact = _prep_inputs(wsi_embeddings, omic_embeddings)
    res = run_bass_kernel_spmd(nc, in_maps, list(range(NCORES)))
    # device columns: per t, 8 relu row-sum cols + 1 diag (S_ii) col;
    # host adds the analytic per-row correction sum_i (1 + MARGIN - 2 d_i)
    grand = float(np.sum(1.0 + MARGIN - 2.0 * d_exact))
    for c in range(NCORES):
        grand += res.results[c]["out"].astype(np.float64).sum()
    return np.float32(grand / (float(N) * float(N)))
